# revision 1
# baseline (speedup 1.0000x reference)
"""Trainium2 Bass kernel for nn_CabbageHeadRefinementLoss.

Self-contained: accepts FULL inputs, shards across 8 NeuronCores internally,
returns the FULL (scalar) output.

Strategy:
  - The O(N^2) ball-query term only involves boundary points
    (0.3 < head_mask_prob < 0.7).  Host compacts those (~3277 of 8192 per
    sample), pads to NBP=4096, and shards rows of the pairwise matrix:
    core c handles sample c//4, rows [1024*(c%4), 1024*(c%4+1)).
  - On device, per core, the pairwise loop runs over 8 rounds x 2 i-chunks,
    each round processing 4 j-chunks CONCURRENTLY via PE tile_position
    packing:
      mm1 (bf16 K=11, 4x row-groups): val[j,i] ~= pj.pi - |pi|^2/2
        - coordinates are split hi/lo into bf16 pairs (x = a + b) so the
          three cross terms a.a + a.b + b.a reproduce fp32 precision to
          ~3e-5 (the dropped b.b term is < 2^-18); fp32 PE matmuls are
          4x slower AND never unthrottle the PE clock (HAM ignores them).
      threshold: ind = (val - |pj|^2/2 > -R2/2), DVE is_gt for even chunks,
        ACT Sign (+-1 with 0.5-scaled weights, corrected on host) for odd.
      mm2 (bf16 K=128, 4x col-groups): acc[32k+c, i] += sum_j ind * v_j[c],
        v = [1, p2, p2^2] (softmax class-2 prob of boundary points).
  - O(N) loss terms (CE/refinement, consistency, pred-head masked moments,
    connectivity distance pass) are reduced on device; sums cross the
    partition dim via a ones-matmul.
  - Host combines: per-row variance math, 3x3 eigendecomposition, gates,
    weighted total.
"""

import numpy as np

try:
    import concourse.bass as bass
except ImportError:  # fallback for environments without NIX_PYTHONPATH
    import sys
    sys.path.insert(0, "/opt/trn_rl_repo")
    import concourse.bass as bass

import concourse.mybir as mybir
import concourse.tile as tile
from concourse import bacc
from concourse.bass_utils import run_bass_kernel_spmd

F32 = mybir.dt.float32
BF16 = mybir.dt.bfloat16
ALU = mybir.AluOpType
ACTF = mybir.ActivationFunctionType

B, N, C = 2, 8192, 3
R2 = np.float32(0.05) * np.float32(0.05)
W_REF, W_CON, W_BND = 0.3, 0.2, 2.0
W_SHP, W_SMO, W_SIZ, W_CNN = 0.5, 0.3, 0.8, 0.6

NBP = 3584          # padded boundary-point count per sample (~7 sigma above
                    # the Binomial(8192, 0.4) boundary-count distribution)
RPC = NBP // 4      # 896 rows per core
FB = NBP // 128     # 28  boundary chunks (also SoA free dim)
FN = N // 128       # 64  full-sample free-dim
ICW = [512, RPC - 512]   # i-chunk widths (ragged second chunk)
NIC = 2
NCORES = 8

_NC_CACHE = None


def _build_nc():
    nc = bacc.Bacc("TRN2", target_bir_lowering=False, debug=False,
                   enable_asserts=False)

    # ---- dram parameters ----
    rbc = nc.dram_tensor("rbc", [96, NBP], BF16, kind="ExternalInput").ap()
    qbc = nc.dram_tensor("qbc", [96, RPC], BF16, kind="ExternalInput").ap()
    pbT = nc.dram_tensor("pbT", [3, NBP], F32, kind="ExternalInput").ap()
    lbT = nc.dram_tensor("lbT", [3, NBP], F32, kind="ExternalInput").ap()
    lgT = nc.dram_tensor("lgT", [3, N], F32, kind="ExternalInput").ap()
    loT = nc.dram_tensor("loT", [3, N], F32, kind="ExternalInput").ap()
    hp = nc.dram_tensor("hp", [N], F32, kind="ExternalInput").ap()
    tg = nc.dram_tensor("tg", [N], F32, kind="ExternalInput").ap()
    ptT = nc.dram_tensor("ptT", [3, N], F32, kind="ExternalInput").ap()

    acc_d = nc.dram_tensor("acc", [3, RPC], F32, kind="ExternalOutput").ap()
    sums_d = nc.dram_tensor("sums", [1, 21], F32, kind="ExternalOutput").ap()
    parts_d = nc.dram_tensor("parts", [128, 1], F32, kind="ExternalOutput").ap()

    with tile.TileContext(nc) as tc:
        with (
            tc.tile_pool(name="const", bufs=1) as const,
            tc.tile_pool(name="work", bufs=8) as work,
            tc.tile_pool(name="tp", bufs=6) as tp,
            tc.tile_pool(name="psA", bufs=3, space="PSUM") as psA,
            tc.tile_pool(name="psB", bufs=1, space="PSUM") as psB,
        ):
            # ---------- loop-critical input DMAs ----------
            # mm1 operands, replicated into the four 32-row groups; only
            # partitions 32k..32k+10 are ever streamed, the gaps stay
            # uninitialized and unread.
            RB = const.tile([96, NBP], BF16)
            nc.sync.dma_start(RB[:], rbc[:])
            QB = const.tile([96, RPC], BF16)
            nc.sync.dma_start(QB[:], qbc[:])
            LB = const.tile([128, 3, FB], F32)
            nc.sync.dma_start(LB[:], lbT.rearrange("c (p f) -> p c f", p=128))
            # boundary coords SoA, natural order: tile (p, c, f) = point
            # p*32+f = mm1 chunk f's output partition p.
            PB = const.tile([128, 3, FB], F32)
            nc.sync.dma_start(PB[:], pbT.rearrange("c (p f) -> p c f", p=128))

            # PE warm-up: dense bf16 K=128 matmuls while input DMAs land.
            # HAM only unthrottles the PE clock after ~3.4us of sustained
            # qualifying work; these fill the otherwise-idle head so the
            # real loop starts (and stays) at 2.4 GHz.
            wz = const.tile([128, 512], BF16)
            nc.vector.memset(wz[:], 1.0)
            wps = psA.tile([128, 1024], F32, tag="d2", name="warm")
            for w in range(24):
                nc.tensor.matmul(wps[:, (w % 2) * 512:(w % 2) * 512 + 512],
                                 wz[:, 0:128], wz[:], start=True, stop=True)

            # ---------- boundary prelude (feeds thresholds + mm2) ----------
            # nrm of boundary points; mh = -nrm/2 ; biasj = -nrm/2 + R2/2
            t0 = work.tile([128, FB], F32)
            nc.vector.tensor_mul(t0[:], PB[:, 0, :], PB[:, 0, :])
            t1 = work.tile([128, FB], F32)
            nc.vector.tensor_mul(t1[:], PB[:, 1, :], PB[:, 1, :])
            t2 = work.tile([128, FB], F32)
            nc.vector.tensor_add(t2[:], t0[:], t1[:])
            t3 = work.tile([128, FB], F32)
            nc.vector.tensor_mul(t3[:], PB[:, 2, :], PB[:, 2, :])
            nrmb = work.tile([128, FB], F32)
            nc.vector.tensor_add(nrmb[:], t2[:], t3[:])
            mh = const.tile([128, FB], F32)
            nc.vector.tensor_scalar(mh[:], nrmb[:], -0.5, None, op0=ALU.mult)
            biasj = const.tile([128, FB], F32)
            nc.vector.tensor_scalar(biasj[:], mh[:], float(R2) / 2.0, None, op0=ALU.add)

            EB = work.tile([128, 3, FB], F32)
            nc.scalar.activation(EB[:], LB[:], ACTF.Exp)
            sB = work.tile([128, FB], F32)
            nc.vector.tensor_add(sB[:], EB[:, 0, :], EB[:, 1, :])
            sB2 = work.tile([128, FB], F32)
            nc.vector.tensor_add(sB2[:], sB[:], EB[:, 2, :])
            rB = work.tile([128, FB], F32)
            nc.vector.reciprocal(rB[:], sB2[:])
            p2b = work.tile([128, FB], F32)
            nc.vector.tensor_mul(p2b[:], EB[:, 2, :], rB[:])

            Vb = const.tile([128, FB, 3], BF16)
            nc.vector.memset(Vb[:, :, 0:1], 1.0)
            nc.vector.tensor_copy(Vb[:, :, 1], p2b[:])
            nc.vector.tensor_mul(Vb[:, :, 2], p2b[:], p2b[:])
            Vh = const.tile([128, FB, 3], BF16)
            nc.vector.tensor_scalar(Vh[:], Vb[:], 0.5, None, op0=ALU.mult)

            # S_odd partials: sum of Vb over odd chunks  -> st2 cols 2:5
            st2 = const.tile([128, 5], F32)
            vodd = Vb.rearrange("p (f2 two) c -> p f2 two c", two=2)[:, :, 1, :]
            nc.vector.tensor_reduce(st2[:, 2:5], vodd.rearrange("p f c -> p c f"),
                                    axis=mybir.AxisListType.X, op=ALU.add)

            # ---------- big pairwise loop: 32 chunks, double-wide tiles ----------
            # d2 tile [128, 1024] spans both i-chunks (2 PSUM banks); one
            # threshold op per chunk; mm2 halves go to col groups 0/1 of a
            # single acc bank (rows 0:3 = i<512, rows 32:35 = i>=512).
            acc_ps = [psB.tile([3, ICW[ic]], F32, tag=f"acc{ic}", name=f"acc_ps{ic}")
                      for ic in range(NIC)]
            thr = float(-R2 / 2.0)
            for f0 in range(FB):
                lR = RB[:, f0 * 128:(f0 + 1) * 128]
                d2 = psA.tile([128, RPC], F32, tag="d2", name="d2")
                for ic in range(NIC):
                    nc.tensor.matmul(d2[:, ic * 512:ic * 512 + ICW[ic]], lR,
                                     QB[:, ic * 512:ic * 512 + ICW[ic]],
                                     start=True, stop=True)
                T = tp.tile([128, RPC], BF16, tag="T", name="T")
                if f0 % 2 == 0:
                    nc.vector.tensor_scalar(T[:], d2[:], mh[:, f0:f0 + 1],
                                            thr, op0=ALU.add, op1=ALU.is_gt)
                else:
                    nc.scalar.activation(T[:], d2[:], ACTF.Sign,
                                         bias=biasj[:, f0:f0 + 1], scale=1.0)
                V = Vb if f0 % 2 == 0 else Vh
                for ic in range(NIC):
                    nc.tensor.matmul(acc_ps[ic][:, 0:ICW[ic]], V[:, f0, :],
                                     T[:, ic * 512:ic * 512 + ICW[ic]],
                                     start=(f0 == 0), stop=(f0 == FB - 1))
            acc_sb = const.tile([3, RPC], F32)
            for ic in range(NIC):
                nc.scalar.copy(acc_sb[:, ic * 512:ic * 512 + ICW[ic]], acc_ps[ic][:])
            nc.sync.dma_start(acc_d[:], acc_sb[:])

            # ---------- full-sample O(N) prelude ----------
            LG = const.tile([128, 3, FN], F32)
            nc.sync.dma_start(LG[:], lgT.rearrange("c (p f) -> p c f", p=128))
            LO = const.tile([128, 3, FN], F32)
            nc.sync.dma_start(LO[:], loT.rearrange("c (p f) -> p c f", p=128))
            PT = const.tile([128, 3, FN], F32)
            nc.sync.dma_start(PT[:], ptT.rearrange("c (p f) -> p c f", p=128))
            HPt = const.tile([128, FN], F32)
            nc.sync.dma_start(HPt[:], hp.rearrange("(p f) -> p f", p=128))
            TGt = const.tile([128, FN], F32)
            nc.sync.dma_start(TGt[:], tg.rearrange("(p f) -> p f", p=128))

            st1 = const.tile([128, 16], F32)
            junk = const.tile([128, FN], F32)
            junk2 = const.tile([128, FN], F32)

            EL = work.tile([128, 3, FN], F32)
            nc.scalar.activation(EL[:], LG[:], ACTF.Exp)
            sl = work.tile([128, FN], F32)
            nc.vector.tensor_add(sl[:], EL[:, 0, :], EL[:, 1, :])
            sl2 = work.tile([128, FN], F32)
            nc.vector.tensor_add(sl2[:], sl[:], EL[:, 2, :])
            rl = work.tile([128, FN], F32)
            nc.vector.reciprocal(rl[:], sl2[:])
            EO = work.tile([128, 3, FN], F32)
            nc.scalar.activation(EO[:], LO[:], ACTF.Exp)
            so = work.tile([128, FN], F32)
            nc.vector.tensor_add(so[:], EO[:, 0, :], EO[:, 1, :])
            so2 = work.tile([128, FN], F32)
            nc.vector.tensor_add(so2[:], so[:], EO[:, 2, :])
            ro = work.tile([128, FN], F32)
            nc.vector.reciprocal(ro[:], so2[:])
            lnS = work.tile([128, FN], F32)
            nc.scalar.activation(lnS[:], sl2[:], ACTF.Ln)

            # consistency: sum over N,C of (softmax(l) - softmax(lo))^2
            for c in range(3):
                pc = work.tile([128, FN], F32, tag="pc", name="pc")
                nc.vector.tensor_mul(pc[:], EL[:, c, :], rl[:])
                qc = work.tile([128, FN], F32, tag="qc", name="qc")
                nc.vector.tensor_mul(qc[:], EO[:, c, :], ro[:])
                dc = work.tile([128, FN], F32, tag="dc", name="dc")
                nc.gpsimd.tensor_sub(dc[:], pc[:], qc[:])
                nc.vector.scalar_tensor_tensor(
                    out=junk2[:], in0=dc[:], scalar=0.0, in1=dc[:],
                    op0=ALU.add, op1=ALU.mult, accum_out=st1[:, 1 + c:2 + c])

            # nll = ln(sum exp) - l[target]
            lt = None
            for c in range(3):
                mc = work.tile([128, FN], F32, tag="mc", name="mc")
                nc.vector.tensor_scalar(mc[:], TGt[:], float(c), None, op0=ALU.is_equal)
                lm = work.tile([128, FN], F32, tag="lm", name="lm")
                nc.gpsimd.tensor_mul(lm[:], LG[:, c, :], mc[:])
                if lt is None:
                    lt = lm
                else:
                    lt2 = work.tile([128, FN], F32, tag="lt2", name="lt2")
                    nc.gpsimd.tensor_add(lt2[:], lt[:], lm[:])
                    lt = lt2
            nll = work.tile([128, FN], F32)
            nc.vector.tensor_sub(nll[:], lnS[:], lt[:])

            # boundary mask, refinement sum = sum (1+bm)*nll
            b1 = work.tile([128, FN], F32)
            nc.vector.tensor_scalar(b1[:], HPt[:], 0.3, None, op0=ALU.is_gt)
            b2 = work.tile([128, FN], F32)
            nc.vector.tensor_scalar(b2[:], HPt[:], 0.7, None, op0=ALU.is_lt)
            bm = work.tile([128, FN], F32)
            nc.vector.tensor_mul(bm[:], b1[:], b2[:])
            nc.vector.tensor_reduce(st1[:, 6:7], bm[:], axis=mybir.AxisListType.X, op=ALU.add)
            nc.vector.scalar_tensor_tensor(
                out=junk[:], in0=bm[:], scalar=1.0, in1=nll[:],
                op0=ALU.add, op1=ALU.mult, accum_out=st1[:, 0:1])

            # pred-head mask m = (l2 > l0) & (l2 > l1)
            g0 = work.tile([128, FN], F32)
            nc.vector.tensor_tensor(g0[:], LG[:, 2, :], LG[:, 0, :], op=ALU.is_gt)
            g1 = work.tile([128, FN], F32)
            nc.vector.tensor_tensor(g1[:], LG[:, 2, :], LG[:, 1, :], op=ALU.is_gt)
            m = const.tile([128, FN], F32)
            nc.gpsimd.tensor_mul(m[:], g0[:], g1[:])
            nc.vector.tensor_reduce(st1[:, 4:5], m[:], axis=mybir.AxisListType.X, op=ALU.add)
            ge2 = work.tile([128, FN], F32)
            nc.vector.tensor_scalar(ge2[:], TGt[:], 2.0, None, op0=ALU.is_equal)
            nc.vector.tensor_reduce(st1[:, 5:6], ge2[:], axis=mybir.AxisListType.X, op=ALU.add)

            # masked moments
            mx = []
            for c in range(3):
                mxc = const.tile([128, FN], F32, tag=f"mx{c}", name=f"mx{c}")
                nc.vector.scalar_tensor_tensor(
                    out=mxc[:], in0=m[:], scalar=0.0, in1=PT[:, c, :],
                    op0=ALU.add, op1=ALU.mult, accum_out=st1[:, 7 + c:8 + c])
                mx.append(mxc)
            pairs = [(0, 0), (1, 1), (2, 2), (0, 1), (0, 2), (1, 2)]
            for kk, (a, bb) in enumerate(pairs):
                eng = nc.vector
                jt = junk2 if kk % 2 == 0 else junk
                eng.scalar_tensor_tensor(
                    out=jt[:], in0=mx[a][:], scalar=0.0, in1=PT[:, bb, :],
                    op0=ALU.add, op1=ALU.mult, accum_out=st1[:, 10 + kk:11 + kk])

            # ones-matmul #1 -> sums1 [1,16]
            ones1 = const.tile([128, 1], F32)
            nc.vector.memset(ones1[:], 1.0)
            sums1 = psA.tile([1, 16], F32, tag="d2", name="sums1")
            nc.tensor.matmul(sums1[:], ones1[:], st1[:], start=True, stop=True)

            # center
            nz = work.tile([1, 1], F32)
            nc.vector.tensor_scalar(nz[:], sums1[0:1, 4:5], 1.0, None, op0=ALU.max)
            rcp = work.tile([1, 1], F32)
            nc.vector.reciprocal(rcp[:], nz[:])
            cen = work.tile([1, 3], F32)
            nc.vector.tensor_scalar(cen[:], sums1[0:1, 7:10], rcp[:], None, op0=ALU.mult)
            sums_sb = const.tile([1, 21], F32)
            nc.vector.tensor_copy(sums_sb[:, 0:16], sums1[:])
            ones2 = const.tile([1, 128], F32)
            nc.vector.memset(ones2[:], 1.0)
            cbp = psA.tile([128, 3], F32, tag="d2", name="cbp")
            nc.tensor.matmul(cbp[:], ones2[:], cen[:], start=True, stop=True)
            cb = const.tile([128, 3], F32)
            nc.vector.tensor_copy(cb[:], cbp[:])

            # distance pass
            dx = work.tile([128, FN], F32)
            nc.vector.tensor_scalar(dx[:], PT[:, 0, :], cb[:, 0:1], None, op0=ALU.subtract)
            dy = work.tile([128, FN], F32)
            nc.vector.tensor_scalar(dy[:], PT[:, 1, :], cb[:, 1:2], None, op0=ALU.subtract)
            dz = work.tile([128, FN], F32)
            nc.vector.tensor_scalar(dz[:], PT[:, 2, :], cb[:, 2:3], None, op0=ALU.subtract)
            s0 = work.tile([128, FN], F32)
            nc.gpsimd.tensor_mul(s0[:], dx[:], dx[:])
            s1t = work.tile([128, FN], F32)
            nc.vector.tensor_mul(s1t[:], dy[:], dy[:])
            s2t = work.tile([128, FN], F32)
            nc.gpsimd.tensor_add(s2t[:], s0[:], s1t[:])
            s3t = work.tile([128, FN], F32)
            nc.vector.tensor_mul(s3t[:], dz[:], dz[:])
            s4t = work.tile([128, FN], F32)
            nc.vector.tensor_add(s4t[:], s2t[:], s3t[:])
            eps12 = const.tile([128, 1], F32)
            nc.vector.memset(eps12[:], 1e-12)
            dd = work.tile([128, FN], F32)
            nc.scalar.activation(dd[:], s4t[:], ACTF.Sqrt, bias=eps12[:, 0:1])
            md = work.tile([128, FN], F32)
            nc.vector.tensor_mul(md[:], m[:], dd[:])
            nc.vector.tensor_reduce(st2[:, 0:1], md[:], axis=mybir.AxisListType.X, op=ALU.add)
            nc.vector.scalar_tensor_tensor(
                out=junk[:], in0=md[:], scalar=0.0, in1=dd[:],
                op0=ALU.add, op1=ALU.mult, accum_out=st2[:, 1:2])
            maxt = const.tile([128, 1], F32)
            nc.vector.tensor_reduce(maxt[:], md[:], axis=mybir.AxisListType.X, op=ALU.max)
            nc.sync.dma_start(parts_d[:], maxt[:])

            # ones-matmul #2 -> sums2 [1,5]
            sums2 = psA.tile([1, 5], F32, tag="d2", name="sums2")
            nc.tensor.matmul(sums2[:], ones1[:], st2[:], start=True, stop=True)
            nc.vector.tensor_copy(sums_sb[:, 16:21], sums2[:])
            nc.sync.dma_start(sums_d[:], sums_sb[:])

    nc.compile()
    return nc


def _get_nc():
    global _NC_CACHE
    if _NC_CACHE is None:
        _NC_CACHE = _build_nc()
    return _NC_CACHE


def _prep_inputs(logits, original_logits, head_mask_prob, targets, points):
    """Build per-core in_maps + host-side row masks."""
    import ml_dtypes
    bf16 = ml_dtypes.bfloat16
    f32 = np.float32
    logits = np.ascontiguousarray(np.asarray(logits, dtype=f32))
    original_logits = np.ascontiguousarray(np.asarray(original_logits, dtype=f32))
    head_mask_prob = np.ascontiguousarray(np.asarray(head_mask_prob, dtype=f32))
    targets_f = np.asarray(targets).astype(f32)
    points = np.ascontiguousarray(np.asarray(points, dtype=f32))

    in_maps = []
    rmasks = []   # per sample: [NBP] bool validity of compacted rows
    for b in range(B):
        hpb = head_mask_prob[b]
        bmask = (hpb > f32(0.3)) & (hpb < f32(0.7))
        idx = np.flatnonzero(bmask)
        nb = idx.size
        assert nb <= NBP, f"boundary count {nb} exceeds padded capacity {NBP}"
        pb = np.full((NBP, 3), f32(100.0))
        pb[:nb] = points[b][idx]
        lb = np.zeros((NBP, 3), f32)
        lb[:nb] = logits[b][idx]
        # mm1 lhsT: comb permutation (column f0*128+p <-> natural point
        # p*32+f0) so each chunk's 128 columns are contiguous; coordinate
        # hi/lo bf16 split: rows [a(3); a(3); b(3); 1; 1]
        pbT = np.ascontiguousarray(pb.T)                      # [3, NBP] natural
        pbT_comb = np.ascontiguousarray(
            pbT.reshape(3, 128, FB).transpose(0, 2, 1).reshape(3, NBP))
        a_c = pbT_comb.astype(bf16)
        b_c = (pbT_comb - a_c.astype(f32)).astype(bf16)
        rbc = np.zeros((96, NBP), bf16)   # K padded to 96: the PE clock only
        rbc[0:3] = a_c                    # unthrottles (HAM) for K > 64
        rbc[3:6] = a_c
        rbc[6:9] = b_c
        rbc[9:11] = np.ones((2, NBP), bf16)
        lbT = np.ascontiguousarray(lb.T)                      # [3, NBP]
        lgT = np.ascontiguousarray(logits[b].T)
        loT = np.ascontiguousarray(original_logits[b].T)
        ptT = np.ascontiguousarray(points[b].T)
        rmasks.append(np.arange(NBP) < nb)
        for s in range(4):
            prT = pb[s * RPC:(s + 1) * RPC].T                 # [3, RPC]
            a_i = prT.astype(bf16)
            b_i = (prT - a_i.astype(f32)).astype(bf16)
            nh = (f32(-0.5) * (prT * prT).sum(0, dtype=f32)).astype(f32)
            nh_a = nh.astype(bf16)
            nh_b = (nh - nh_a.astype(f32)).astype(bf16)
            qbc = np.zeros((96, RPC), bf16)
            qbc[0:3] = a_i
            qbc[3:6] = b_i
            qbc[6:9] = a_i
            qbc[9] = nh_a
            qbc[10] = nh_b
            in_maps.append({
                "lgT": lgT, "loT": loT, "hp": hpb, "tg": targets_f[b],
                "ptT": ptT, "pbT": pbT, "lbT": lbT,
                "rbc": rbc, "qbc": qbc,
            })
    return in_maps, rmasks


def _postprocess(results, rmasks):
    totals = []
    for b in range(B):
        S = results[4 * b]["sums"][0].astype(np.float64)
        acc = np.concatenate(
            [results[4 * b + s]["acc"] for s in range(4)], axis=1
        ).astype(np.float64)                                   # [3, NBP]
        # column layout: 0 nllw | 1:4 cons_c | 4 n_pred | 5 n_gt | 6 bm_sum |
        #                7:10 Smx | 10:16 M2 | 16 Smd | 17 Smd2 | 18:21 S_odd
        corr = 0.5 * S[18:21]
        cnt = acc[0] + corr[0]
        s1 = acc[1] + corr[1]
        s2 = acc[2] + corr[2]
        var = (s2 - s1 * s1 / np.maximum(cnt, 1.0)) / np.maximum(cnt - 1.0, 1.0)
        valid = rmasks[b] & (cnt > 1.0)
        bm_sum = S[6]
        smooth = (var * valid).sum() / max(valid.sum(), 1.0) if bm_sum >= 5.0 else 0.0

        refinement = S[0] / N
        consistency = (S[1] + S[2] + S[3]) / (N * C)
        n, ngt = S[4], S[5]
        nz = max(n, 1.0)
        Sx = S[7:10]
        M2 = np.array([[S[10], S[13], S[14]],
                       [S[13], S[11], S[15]],
                       [S[14], S[15], S[12]]])
        cen = Sx / nz
        cov = (M2 - np.outer(cen, Sx) - np.outer(Sx, cen) + n * np.outer(cen, cen)) / nz
        if n >= 10.0:
            ev = np.linalg.eigvalsh(cov)
            a = ev[2]
            shape = (ev[1] / (a + 1e-8) - 1.0) ** 2 + (ev[0] / (a + 1e-8) - 1.0) ** 2
        else:
            shape = 0.0
        mean_d = S[16] / nz
        var_d = (S[17] - 2.0 * mean_d * S[16] + mean_d * mean_d * n) / max(n - 1.0, 1.0)
        max_d = float(results[4 * b]["parts"].max())
        conn = var_d / (max_d + 1e-8) if n >= 5.0 else 0.0
        vol = (n - ngt) ** 2
        rel = abs(n - ngt) / max(ngt, 1.0)
        size = vol + 0.5 * rel if ngt > 0.0 else vol

        geometric = W_SHP * shape + W_SMO * smooth + W_SIZ * size + W_CNN * conn
        totals.append(W_REF * refinement + W_CON * consistency + geometric)
    return np.float32(np.mean(totals))


def run(trace=False, **inputs):
    """Run the kernel; returns (output_scalar, BassKernelResults)."""
    nc = _get_nc()
    in_maps, rmasks = _prep_inputs(**inputs)
    res = run_bass_kernel_spmd(nc, in_maps, core_ids=list(range(NCORES)),
                               trace=trace)
    out = _postprocess(res.results, rmasks)
    return out, res


def kernel(logits, original_logits, head_mask_prob, targets, points):
    out, _ = run(logits=logits, original_logits=original_logits,
                 head_mask_prob=head_mask_prob, targets=targets, points=points)
    return out



# revision 3
# speedup vs baseline: 1.0184x; 1.0184x over previous
"""Trainium2 Bass kernel for nn_CabbageHeadRefinementLoss — pruned redesign.

Self-contained: accepts FULL inputs, shards across 8 NeuronCores internally,
returns the FULL (scalar) output.

Strategy (v2 — x-sort pruned ball query):
  - Boundary points (~3250 of 8192) are compacted AND SORTED BY X on host,
    padded to NBP=3584.  Each core owns an i-slab of 896 sorted ranks; only
    j's within PAD=256 ranks of the slab can be within R=0.05 in x (host
    asserts the true rank window <= PAD), so each core processes an 11-chunk
    j-window of 1408 ranks instead of all 3584: 4480 matmul columns/core
    instead of 25088 (5.6x less PE work).
  - Per j-chunk k (128 j's), i-columns C_k = [128k-512, 128k+128) ∩ [0,896):
    mm1 (bf16 hi/lo split, K=11) -> d2 PSUM; threshold alternates DVE is_gt
    (even k) / ACT Sign with half-weights (odd k, corrected on host); mm2
    accumulates [1, p2, p2^2] per 128-col block with exact start/stop flags.
  - No PE warm-up: total matmul work (~9k cycles) is below the HAM unthrottle
    threshold, so the loop targets the steady 1.2 GHz p-state.
  - O(N) terms: refinement/consistency/target-counts sharded in quarters
    across the 4 cores of a sample (bf16 inputs); pred-head mask computed on
    host (must be exact); moments + center + distance pass replicated.
    sqrt(s) computed as exp(0.5*ln(s)) so only ONE ACT table set (id 6:
    exp/ln/sign/copy) is ever loaded.
  - All inputs packed into 3 per-partition-contiguous DMA blobs; outputs are
    row-contiguous (no [128,1]-shaped DMAs -> no completion-counter straggler).
  - Host combines: per-row variance, 3x3 eigendecomposition, gates, total.
"""

import numpy as np

try:
    import concourse.bass as bass
except ImportError:  # fallback for environments without NIX_PYTHONPATH
    import sys
    sys.path.insert(0, "/opt/trn_rl_repo")
    import concourse.bass as bass

import concourse.mybir as mybir
import concourse.tile as tile
from concourse import bacc
from concourse.bass_utils import run_bass_kernel_spmd

F32 = mybir.dt.float32
BF16 = mybir.dt.bfloat16
ALU = mybir.AluOpType
ACTF = mybir.ActivationFunctionType

B, N, C = 2, 8192, 3
R2 = np.float32(0.05) * np.float32(0.05)
W_REF, W_CON, W_BND = 0.3, 0.2, 2.0
W_SHP, W_SMO, W_SIZ, W_CNN = 0.5, 0.3, 0.8, 0.6

NBP = 3584
SLAB = 896          # i-ranks per core
PAD = 256           # j rank-window halo (host asserts data fits)
JW = SLAB + 2 * PAD  # 1408 j-window
NCH = JW // 128      # 11 j-chunks
NBLK = SLAB // 128   # 7 acc column blocks
QN = N // 4          # 2048 per-core quarter for sharded O(N) terms
FQ = QN // 128       # 16
FN = N // 128        # 64
NCORES = 8

SIGN_CHUNKS = (1, 3, 5, 7, 9)   # chunks thresholded via ACT Sign (half-weights)

# sums row layout (host side):
#   [0]=n  [1:4]=Smx  [4:10]=M2(xx,yy,zz,xy,xz,yz)          <- ones-matmul #1
#   [10]=nllw_q [11:14]=cons_c [14]=ngt_q [15]=bm_sum_q
#   [16]=Smd_q [17]=Smd2_q [18:33]=vodd(k in SIGN_CHUNKS)   <- ones-matmul #2
#   [33]=maxd_q

_NC_CACHE = None


def _chunk_cols(k):
    return max(0, 128 * k - 2 * PAD), min(SLAB, 128 * k + 128)


def _blk_range(k):
    return max(0, k - 4), min(NBLK - 1, k)


def _build_nc():
    nc = bacc.Bacc("TRN2", target_bir_lowering=False, debug=False,
                   enable_asserts=False)

    rq_d = nc.dram_tensor("rq", [11, JW + SLAB], BF16, kind="ExternalInput").ap()
    hb_d = nc.dram_tensor("hb", [128, 512], BF16, kind="ExternalInput").ap()
    acc_d = nc.dram_tensor("acc", [3, SLAB], F32, kind="ExternalOutput").ap()
    sums_d = nc.dram_tensor("sums", [1, 64], F32, kind="ExternalOutput").ap()

    thr = float(-R2 / 2.0)

    with tile.TileContext(nc) as tc:
        with (
            tc.tile_pool(name="const", bufs=1) as const,
            tc.tile_pool(name="work", bufs=8) as work,
            tc.tile_pool(name="tp", bufs=4) as tp,
            tc.tile_pool(name="psD", bufs=2, space="PSUM") as psD,
            tc.tile_pool(name="psS", bufs=1, space="PSUM") as psS,
            tc.tile_pool(name="psA", bufs=1, space="PSUM") as psA,
        ):
            # Load ACT set 6 (exp+ln+sign+copy) once, up front: the greedy
            # table-load pass would otherwise thrash between sets 0 and 5
            # (1.28us per reload on the Scalar critical path).
            _li = mybir.InstLoadActFuncSet(
                name=nc.get_next_instruction_name(), ins=[], outs=[],
                act_func_set_id=6)
            nc.scalar.add_instruction(_li)

            # ---------- input DMAs ----------
            RQ = const.tile([11, JW + SLAB], BF16)
            nc.sync.dma_start(RQ[:], rq_d[:])
            H = const.tile([128, 512], BF16)
            nc.gpsimd.dma_start(H[:], hb_d[:])

            RB = RQ[:, 0:JW]
            QB = RQ[:, JW:JW + SLAB]
            lbw = H[:, 448:481].rearrange("p (k c) -> p k c", c=3)
            with tc.high_priority():
                mh_t = const.tile([128, 11], F32)
                nc.vector.tensor_add(mh_t[:], H[:, 481:492], H[:, 492:503])
                biasj_t = const.tile([128, 11], F32)
                nc.vector.tensor_scalar(biasj_t[:], mh_t[:], float(R2) / 2.0,
                                        None, op0=ALU.add)
            mh = mh_t
            biasj = biasj_t
            lgq = H[:, 0:48].rearrange("p (c f) -> p c f", c=3)
            loq = H[:, 48:96].rearrange("p (c f) -> p c f", c=3)
            tgq = H[:, 96:112]
            bmq = H[:, 112:128]
            m_f = H[:, 128:192]
            ptT = H[:, 192:384].rearrange("p (c f) -> p c f", c=3)
            m_q = H[:, 384:400]
            ptq = H[:, 400:448].rearrange("p (c f) -> p c f", c=3)

            stA = const.tile([128, 10], F32)   # n, Smx, M2
            st2 = const.tile([128, 23], F32)   # nllw, cons, ge2, bm_sum, Smd,
            junk = const.tile([128, FN], F32)  # Smd2, vodd
            junk2 = const.tile([128, FN], F32)

            acc_ps = psA.tile([3, SLAB], F32, tag="accp", name="acc_ps")
            with tc.high_priority():
                nc.vector.memset(acc_ps[:], 0.0)

            def d2t(k):
                lo, hi = _chunk_cols(k)
                w = hi - lo
                lhsT = RB[:, 128 * k:128 * (k + 1)]
                with tc.high_priority():
                    d2 = psD.tile([128, 640], F32, tag="d2", name=f"d2_{k}")
                    for c0 in range(0, w, 512):
                        c1 = min(c0 + 512, w)
                        nc.tensor.matmul(d2[:, c0:c1], lhsT,
                                         QB[:, lo + c0:lo + c1],
                                         start=True, stop=True)
                    T = tp.tile([128, 640], BF16, tag="T", name=f"T_{k}")
                    if k not in SIGN_CHUNKS:
                        nc.vector.tensor_scalar(T[:, 0:w], d2[:, 0:w],
                                                mh[:, k:k + 1], thr,
                                                op0=ALU.add, op1=ALU.is_gt)
                    else:
                        nc.scalar.activation(T[:, 0:w], d2[:, 0:w], ACTF.Sign,
                                             bias=biasj[:, k:k + 1], scale=1.0)
                return T

            def accm(k, T):
                lo, hi = _chunk_cols(k)
                w = hi - lo
                V = Vh if k in SIGN_CHUNKS else Vb
                with tc.high_priority():
                    # accumulating matmul (PSUM pre-zeroed by the memset)
                    for c0 in range(0, w, 512):
                        c1 = min(c0 + 512, w)
                        nc.tensor.matmul(acc_ps[:, lo + c0:lo + c1],
                                         V[:, k, :], T[:, c0:c1],
                                         start=False, stop=False,
                                         skip_group_check=True)

            def chunk(k):
                accm(k, d2t(k))

            # ---------- chunks 0-1: thresholds first, V build, then mm2 ----------
            T0 = d2t(0)
            T1 = d2t(1)

            # ---------- boundary p2 / V (dep: H) ----------
            with tc.high_priority():
                ELB = work.tile([128, 11, 3], F32)
                nc.scalar.activation(ELB[:], lbw, ACTF.Exp)
                sb = work.tile([128, 11], F32)
                nc.vector.tensor_add(sb[:], ELB[:, :, 0], ELB[:, :, 1])
                sb2 = work.tile([128, 11], F32)
                nc.vector.tensor_add(sb2[:], sb[:], ELB[:, :, 2])
                rb = work.tile([128, 11], F32)
                nc.vector.reciprocal(rb[:], sb2[:])
                p2 = work.tile([128, 11], F32)
                nc.vector.tensor_mul(p2[:], ELB[:, :, 2], rb[:])
                Vb = const.tile([128, 11, 3], BF16)
                nc.vector.memset(Vb[:, :, 0:1], 1.0)
                nc.vector.tensor_copy(Vb[:, :, 1], p2[:])
                nc.vector.tensor_mul(Vb[:, :, 2], p2[:], p2[:])
                Vh = const.tile([128, 11, 3], BF16)
                nc.vector.tensor_scalar(Vh[:], Vb[:], 0.5, None, op0=ALU.mult)

            accm(0, T0)
            accm(1, T1)


            # ---------- O(N) stage A: count + moments (dep: H) ----------
            nc.vector.tensor_reduce(stA[:, 0:1], m_f, axis=mybir.AxisListType.X,
                                    op=ALU.add)
            mx = []
            for c in range(3):
                mxc = const.tile([128, FN], F32, tag=f"mx{c}", name=f"mx{c}")
                nc.vector.scalar_tensor_tensor(
                    out=mxc[:], in0=m_f, scalar=0.0, in1=ptT[:, c, :],
                    op0=ALU.add, op1=ALU.mult, accum_out=stA[:, 1 + c:2 + c])
                mx.append(mxc)
            pairs = [(0, 0), (1, 1), (2, 2), (0, 1), (0, 2), (1, 2)]
            for kk, (a, bb) in enumerate(pairs):
                nc.vector.scalar_tensor_tensor(
                    out=junk2[:], in0=mx[a][:], scalar=0.0, in1=ptT[:, bb, :],
                    op0=ALU.add, op1=ALU.mult, accum_out=stA[:, 4 + kk:5 + kk])

            chunk(2)
            chunk(3)

            # ---------- center chain ----------
            ones1 = const.tile([128, 1], F32)
            nc.vector.memset(ones1[:], 1.0)
            sums1 = psS.tile([1, 10], F32, tag="s1", name="sums1")
            nc.tensor.matmul(sums1[:], ones1[:], stA[:], start=True, stop=True)
            nz = work.tile([1, 1], F32)
            nc.vector.tensor_scalar(nz[:], sums1[0:1, 0:1], 1.0, None, op0=ALU.max)
            rcp = work.tile([1, 1], F32)
            nc.vector.reciprocal(rcp[:], nz[:])
            cen = work.tile([1, 3], F32)
            nc.vector.tensor_scalar(cen[:], sums1[0:1, 1:4], rcp[:], None,
                                    op0=ALU.mult)
            sums_sb = const.tile([1, 64], F32)
            nc.vector.tensor_copy(sums_sb[:, 0:10], sums1[:])
            ones2 = const.tile([1, 128], F32)
            nc.vector.memset(ones2[:], 1.0)
            cbp = psS.tile([128, 3], F32, tag="s1", name="cbp")
            nc.tensor.matmul(cbp[:], ones2[:], cen[:], start=True, stop=True)
            cb = const.tile([128, 3], F32)
            nc.vector.tensor_copy(cb[:], cbp[:])

            chunk(4)

            # ---------- O(N) stage B: softmax / consistency / nll (quarter) ----------
            EL = work.tile([128, 3, FQ], F32)
            nc.scalar.activation(EL[:], lgq, ACTF.Exp)
            sl = work.tile([128, FQ], F32)
            nc.vector.tensor_add(sl[:], EL[:, 0, :], EL[:, 1, :])
            sl2 = work.tile([128, FQ], F32)
            nc.vector.tensor_add(sl2[:], sl[:], EL[:, 2, :])
            rl = work.tile([128, FQ], F32)
            nc.vector.reciprocal(rl[:], sl2[:])
            EO = work.tile([128, 3, FQ], F32)
            nc.scalar.activation(EO[:], loq, ACTF.Exp)
            so = work.tile([128, FQ], F32)
            nc.gpsimd.tensor_add(so[:], EO[:, 0, :], EO[:, 1, :])
            so2 = work.tile([128, FQ], F32)
            nc.gpsimd.tensor_add(so2[:], so[:], EO[:, 2, :])
            ro = work.tile([128, FQ], F32)
            nc.vector.reciprocal(ro[:], so2[:])
            lnS = work.tile([128, FQ], F32)
            nc.scalar.activation(lnS[:], sl2[:], ACTF.Ln)

            chunk(5)

            for c in range(3):
                pc = work.tile([128, FQ], F32, tag="pc", name="pc")
                nc.vector.tensor_mul(pc[:], EL[:, c, :], rl[:])
                qc = work.tile([128, FQ], F32, tag="qc", name="qc")
                nc.gpsimd.tensor_mul(qc[:], EO[:, c, :], ro[:])
                dc = work.tile([128, FQ], F32, tag="dc", name="dc")
                nc.gpsimd.tensor_sub(dc[:], pc[:], qc[:])
                nc.vector.scalar_tensor_tensor(
                    out=junk2[:, 0:FQ], in0=dc[:], scalar=0.0, in1=dc[:],
                    op0=ALU.add, op1=ALU.mult, accum_out=st2[:, 1 + c:2 + c])

            lt = None
            for c in range(3):
                mc = work.tile([128, FQ], F32, tag=f"mc{c}", name=f"mc{c}")
                nc.vector.tensor_scalar(mc[:], tgq, float(c), None,
                                        op0=ALU.is_equal)
                lm = work.tile([128, FQ], F32, tag="lm", name="lm")
                nc.gpsimd.tensor_mul(lm[:], lgq[:, c, :], mc[:])
                if lt is None:
                    lt = lm
                else:
                    lt2 = work.tile([128, FQ], F32, tag="lt2", name="lt2")
                    nc.gpsimd.tensor_add(lt2[:], lt[:], lm[:])
                    lt = lt2
                if c == 2:
                    nc.vector.tensor_reduce(st2[:, 4:5], mc[:],
                                            axis=mybir.AxisListType.X, op=ALU.add)
            nll = work.tile([128, FQ], F32)
            nc.vector.tensor_sub(nll[:], lnS[:], lt[:])
            nc.vector.scalar_tensor_tensor(
                out=junk2[:, 0:FQ], in0=bmq, scalar=1.0, in1=nll[:],
                op0=ALU.add, op1=ALU.mult, accum_out=st2[:, 0:1])
            nc.vector.tensor_reduce(st2[:, 5:6], bmq, axis=mybir.AxisListType.X,
                                    op=ALU.add)

            chunk(6)

            # ---------- distance pass (replicated full sample) ----------
            dx = work.tile([128, FQ], F32)
            nc.vector.tensor_scalar(dx[:], ptq[:, 0, :], cb[:, 0:1], None,
                                    op0=ALU.subtract)
            dy = work.tile([128, FQ], F32)
            nc.vector.tensor_scalar(dy[:], ptq[:, 1, :], cb[:, 1:2], None,
                                    op0=ALU.subtract)
            dz = work.tile([128, FQ], F32)
            nc.vector.tensor_scalar(dz[:], ptq[:, 2, :], cb[:, 2:3], None,
                                    op0=ALU.subtract)
            s0 = work.tile([128, FQ], F32)
            nc.gpsimd.tensor_mul(s0[:], dx[:], dx[:])
            s1t = work.tile([128, FQ], F32)
            nc.vector.tensor_mul(s1t[:], dy[:], dy[:])
            s2t = work.tile([128, FQ], F32)
            nc.gpsimd.tensor_add(s2t[:], s0[:], s1t[:])
            s3t = work.tile([128, FQ], F32)
            nc.gpsimd.tensor_mul(s3t[:], dz[:], dz[:])
            s4t = work.tile([128, FQ], F32)
            nc.vector.tensor_add(s4t[:], s2t[:], s3t[:])
            eps12 = const.tile([128, 1], F32)
            nc.vector.memset(eps12[:], 1e-12)
            ls = work.tile([128, FQ], F32)
            nc.scalar.activation(ls[:], s4t[:], ACTF.Ln, bias=eps12[:, 0:1])
            dd = work.tile([128, FQ], F32)
            nc.scalar.activation(dd[:], ls[:], ACTF.Exp, bias=0.0, scale=0.5)
            md = work.tile([128, FQ], F32)
            nc.gpsimd.tensor_mul(md[:], m_q, dd[:])
            nc.vector.tensor_reduce(st2[:, 6:7], md[:], axis=mybir.AxisListType.X,
                                    op=ALU.add)
            nc.vector.scalar_tensor_tensor(
                out=junk2[:, 0:FQ], in0=md[:], scalar=0.0, in1=dd[:],
                op0=ALU.add, op1=ALU.mult, accum_out=st2[:, 7:8])
            maxsc = work.tile([1, 1], F32)
            nc.gpsimd.tensor_reduce(maxsc[:], md[:], axis=mybir.AxisListType.XYZWC,
                                    op=ALU.max)
            nc.vector.tensor_copy(sums_sb[:, 33:34], maxsc[:])

            chunk(7)

            # vodd copies for host Sign correction
            for i, ko in enumerate(SIGN_CHUNKS):
                nc.gpsimd.tensor_copy(st2[:, 8 + 3 * i:11 + 3 * i], Vb[:, ko, :])

            chunk(8)

            # acc cols [0:640] complete after chunk 8 (contributors k<=8)
            acc_sb = const.tile([3, SLAB], F32)
            nc.scalar.copy(acc_sb[:, 0:640], acc_ps[:, 0:640])
            nc.sync.dma_start(acc_d[:, 0:640], acc_sb[:, 0:640])

            chunk(9)

            sums2 = psS.tile([1, 23], F32, tag="s1", name="sums2")
            nc.tensor.matmul(sums2[:], ones1[:], st2[:], start=True, stop=True)
            nc.vector.tensor_copy(sums_sb[:, 10:33], sums2[:])
            nc.sync.dma_start(sums_d[:], sums_sb[:])
            nc.vector.tensor_copy(acc_sb[:, 640:768], acc_ps[:, 640:768])

            chunk(10)

            nc.vector.tensor_copy(acc_sb[:, 768:SLAB], acc_ps[:, 768:SLAB])
            nc.sync.dma_start(acc_d[:, 640:SLAB], acc_sb[:, 640:SLAB])

    nc.compile()
    return nc


def _get_nc():
    global _NC_CACHE
    if _NC_CACHE is None:
        _NC_CACHE = _build_nc()
    return _NC_CACHE


def _prep_inputs(logits, original_logits, head_mask_prob, targets, points):
    import ml_dtypes
    bf16 = ml_dtypes.bfloat16
    f32 = np.float32
    logits = np.ascontiguousarray(np.asarray(logits, dtype=f32))
    original_logits = np.ascontiguousarray(np.asarray(original_logits, dtype=f32))
    head_mask_prob = np.ascontiguousarray(np.asarray(head_mask_prob, dtype=f32))
    targets_f = np.asarray(targets).astype(f32)
    points = np.ascontiguousarray(np.asarray(points, dtype=f32))

    in_maps = []
    recon = []
    for b in range(B):
        hp = head_mask_prob[b]
        bmask = (hp > f32(0.3)) & (hp < f32(0.7))
        idx = np.flatnonzero(bmask)
        nb = idx.size
        assert nb <= NBP, f"boundary count {nb} exceeds {NBP}"
        pts = points[b][idx]
        order = np.argsort(pts[:, 0], kind="stable")
        pts_s = np.full((NBP, 3), f32(100.0))
        pts_s[:nb] = pts[order]
        lgs_s = np.zeros((NBP, 3), f32)
        lgs_s[:nb] = logits[b][idx][order]
        ptE = np.concatenate([
            np.full((PAD, 3), f32(-1000.0)), pts_s,
            np.full((PAD, 3), f32(2000.0))])
        lgE = np.concatenate([
            np.zeros((PAD, 3), f32), lgs_s, np.zeros((PAD, 3), f32)])

        xs = pts_s[:nb, 0]
        lo = np.searchsorted(xs, xs - f32(0.051), side="left")
        hi = np.searchsorted(xs, xs + f32(0.051), side="right")
        Wmax = max((np.arange(nb) - lo).max(), (hi - 1 - np.arange(nb)).max())
        assert Wmax <= PAD, f"rank window {Wmax} exceeds PAD={PAD}"

        recon.append(dict(nb=nb))

        lg = logits[b]
        m_full = ((lg[:, 2] > lg[:, 0]) & (lg[:, 2] > lg[:, 1])).astype(f32)

        for s in range(4):
            pi = pts_s[SLAB * s: SLAB * (s + 1)]
            a_i = pi.T.astype(bf16)
            b_i = (pi.T - a_i.astype(f32)).astype(bf16)
            nh = (f32(-0.5) * (pi * pi).sum(1, dtype=f32)).astype(f32)
            nh_a = nh.astype(bf16)
            nh_b = (nh - nh_a.astype(f32)).astype(bf16)
            pj = ptE[SLAB * s: SLAB * s + JW]
            a_j = pj.T.astype(bf16)
            b_j = (pj.T - a_j.astype(f32)).astype(bf16)
            rq = np.zeros((11, JW + SLAB), bf16)
            rq[0:3, 0:JW] = a_j
            rq[3:6, 0:JW] = a_j
            rq[6:9, 0:JW] = b_j
            rq[9:11, 0:JW] = np.ones((2, JW), bf16)
            rq[0:3, JW:] = a_i
            rq[3:6, JW:] = b_i
            rq[6:9, JW:] = a_i
            rq[9, JW:] = nh_a
            rq[10, JW:] = nh_b

            nrm_j = (pj * pj).sum(1, dtype=f32)
            mh_v = (f32(-0.5) * nrm_j).reshape(NCH, 128).T.astype(f32)
            mh_hi = mh_v.astype(bf16)
            mh_lo = (mh_v - mh_hi.astype(f32)).astype(bf16)
            lbw_v = lgE[SLAB * s: SLAB * s + JW].reshape(
                NCH, 128, 3).transpose(1, 0, 2).reshape(128, 33)

            q0 = QN * s
            hbl = np.zeros((128, 512), bf16)
            hbl[:, 448:481] = lbw_v.astype(bf16)
            hbl[:, 481:492] = mh_hi
            hbl[:, 492:503] = mh_lo
            hbl[:, 0:48] = logits[b][q0:q0 + QN].reshape(128, FQ, 3).transpose(
                0, 2, 1).reshape(128, 48).astype(bf16)
            hbl[:, 48:96] = original_logits[b][q0:q0 + QN].reshape(
                128, FQ, 3).transpose(0, 2, 1).reshape(128, 48).astype(bf16)
            hbl[:, 96:112] = targets_f[b][q0:q0 + QN].reshape(128, FQ).astype(bf16)
            hbl[:, 112:128] = bmask[q0:q0 + QN].astype(f32).reshape(
                128, FQ).astype(bf16)
            hbl[:, 128:192] = m_full.reshape(128, FN).astype(bf16)
            hbl[:, 192:384] = points[b].reshape(128, FN, 3).transpose(
                0, 2, 1).reshape(128, 192).astype(bf16)
            hbl[:, 384:400] = m_full[q0:q0 + QN].reshape(128, FQ).astype(bf16)
            hbl[:, 400:448] = points[b][q0:q0 + QN].reshape(128, FQ, 3).transpose(
                0, 2, 1).reshape(128, 48).astype(bf16)

            in_maps.append({"rq": rq, "hb": hbl})
    return in_maps, recon


def _postprocess(results, recon):
    totals = []
    for b in range(B):
        outs = results[4 * b:4 * b + 4]
        nb = recon[b]["nb"]
        S = [o["sums"][0].astype(np.float64) for o in outs]
        acc = np.concatenate([o["acc"] for o in outs], axis=1).astype(np.float64)
        corr = np.zeros((NBP, 3))
        for s in range(4):
            for i, ko in enumerate(SIGN_CHUNKS):
                lo, hi = _chunk_cols(ko)
                corr[SLAB * s + lo: SLAB * s + hi] += 0.5 * S[s][18 + 3 * i:21 + 3 * i]
        cnt = acc[0] + corr[:, 0]
        s1 = acc[1] + corr[:, 1]
        s2 = acc[2] + corr[:, 2]
        var = (s2 - s1 * s1 / np.maximum(cnt, 1.0)) / np.maximum(cnt - 1.0, 1.0)
        valid = (np.arange(NBP) < nb) & (cnt > 1.0)
        bm_sum = sum(Sx[15] for Sx in S)
        smooth = (var * valid).sum() / max(valid.sum(), 1.0) if bm_sum >= 5.0 else 0.0

        refinement = sum(Sx[10] for Sx in S) / N
        consistency = sum(Sx[11] + Sx[12] + Sx[13] for Sx in S) / (N * C)
        S0 = S[0]
        n = S0[0]
        ngt = sum(Sx[14] for Sx in S)
        nz = max(n, 1.0)
        Sx_ = S0[1:4]
        M2 = np.array([[S0[4], S0[7], S0[8]],
                       [S0[7], S0[5], S0[9]],
                       [S0[8], S0[9], S0[6]]])
        cen = Sx_ / nz
        cov = (M2 - np.outer(cen, Sx_) - np.outer(Sx_, cen)
               + n * np.outer(cen, cen)) / nz
        if n >= 10.0:
            ev = np.linalg.eigvalsh(cov)
            a = ev[2]
            shape = (ev[1] / (a + 1e-8) - 1.0) ** 2 + (ev[0] / (a + 1e-8) - 1.0) ** 2
        else:
            shape = 0.0
        Smd = sum(Sx[16] for Sx in S)
        Smd2 = sum(Sx[17] for Sx in S)
        mean_d = Smd / nz
        var_d = (Smd2 - 2.0 * mean_d * Smd + mean_d * mean_d * n) / max(n - 1.0, 1.0)
        max_d = max(Sx[33] for Sx in S)
        conn = var_d / (max_d + 1e-8) if n >= 5.0 else 0.0
        vol = (n - ngt) ** 2
        rel = abs(n - ngt) / max(ngt, 1.0)
        size = vol + 0.5 * rel if ngt > 0.0 else vol

        geometric = W_SHP * shape + W_SMO * smooth + W_SIZ * size + W_CNN * conn
        totals.append(W_REF * refinement + W_CON * consistency + geometric)
    return np.float32(np.mean(totals))


def run(trace=False, **inputs):
    nc = _get_nc()
    in_maps, recon = _prep_inputs(**inputs)
    res = run_bass_kernel_spmd(nc, in_maps, core_ids=list(range(NCORES)),
                               trace=trace)
    out = _postprocess(res.results, recon)
    return out, res


def kernel(logits, original_logits, head_mask_prob, targets, points):
    out, _ = run(logits=logits, original_logits=original_logits,
                 head_mask_prob=head_mask_prob, targets=targets, points=points)
    return out
